# revision 3
# baseline (speedup 1.0000x reference)
"""Distributed 2-layer GraphSAGE (mean aggr) + linear head on 8 NeuronCores.

Sharding (per the standard 1D graph-partition recipe):
  - nodes are sharded 8 ways by contiguous ranges of 12500 (dst ownership)
  - edges are partitioned by dst shard on the host (pure data layout)
  - x and the small weight matrices are replicated on every core
  - layer-1 hidden states are all-gathered between the layers so every core
    can gather source features for its own edges
  - every core computes the full pipeline for its 12500 nodes; outputs are
    concatenated back to the full [100000, 16] result.

Self-contained: hardcodes N=100000, E=1600000, D=128, 8 cores.
"""
import numpy as np
import jax
import jax.numpy as jnp
from jax.sharding import Mesh, PartitionSpec as P
from jax.experimental.shard_map import shard_map
from functools import partial

N = 100000
E = 1600000
D = 128
CORES = 8
NSH = N // CORES  # 12500


def _partition_edges(edge_index: np.ndarray):
    """Bucket edges by dst shard; pad each bucket to a common length."""
    src = edge_index[0].astype(np.int32)
    dst = edge_index[1].astype(np.int32)
    order = np.argsort(dst, kind="stable")  # sorted by dst -> shards contiguous
    src_s, dst_s = src[order], dst[order]
    counts = np.bincount(dst_s // NSH, minlength=CORES)
    emax = int(((counts.max() + 127) // 128) * 128)
    src_p = np.zeros((CORES, emax), np.int32)
    dstl_p = np.zeros((CORES, emax), np.int32)
    w_p = np.zeros((CORES, emax), np.float32)
    off = 0
    for c in range(CORES):
        n = int(counts[c])
        src_p[c, :n] = src_s[off:off + n]
        dstl_p[c, :n] = dst_s[off:off + n] - c * NSH
        w_p[c, :n] = 1.0
        off += n
    return src_p, dstl_p, w_p


def _sage_local(x_full, x_own, src, dstl, w, Wl, bl, Wr, chunks=4):
    ec = src.shape[0] // chunks
    summed = jnp.zeros((NSH, D), jnp.float32)
    cnt = jnp.zeros((NSH,), jnp.float32)
    for i in range(chunks):
        s = jax.lax.slice_in_dim(src, i * ec, (i + 1) * ec)
        d = jax.lax.slice_in_dim(dstl, i * ec, (i + 1) * ec)
        ww = jax.lax.slice_in_dim(w, i * ec, (i + 1) * ec)
        msg = jnp.take(x_full, s, axis=0) * ww[:, None]
        summed = summed + jax.ops.segment_sum(
            msg, d, num_segments=NSH, indices_are_sorted=True)
        cnt = cnt + jax.ops.segment_sum(
            ww, d, num_segments=NSH, indices_are_sorted=True)
    agg = summed / jnp.maximum(cnt, 1.0)[:, None]
    return agg @ Wl.T + bl + x_own @ Wr.T


def _body(xf, x_own, src, dstl, w, W1l, b1l, W1r, W2l, b2l, W2r, Wc, bc):
    src = src.reshape(-1)
    dstl = dstl.reshape(-1)
    w = w.reshape(-1)
    x_own = x_own.reshape(NSH, D)
    h = jax.nn.relu(_sage_local(xf, x_own, src, dstl, w, W1l, b1l, W1r))
    hf = jax.lax.all_gather(h, "core", tiled=True)  # [N, D]
    z = _sage_local(hf, h, src, dstl, w, W2l, b2l, W2r)
    out = z @ Wc.T + bc
    return out


_cache = {}


def _compiled():
    if "fn" not in _cache:
        mesh = Mesh(np.asarray(jax.devices()[:CORES]), ("core",))
        rep = P()
        shd = P("core")
        fn = jax.jit(shard_map(
            _body, mesh=mesh,
            in_specs=(rep, shd, shd, shd, shd, rep, rep, rep, rep, rep, rep, rep, rep),
            out_specs=shd,
            check_rep=False,
        ))
        _cache["fn"] = fn
        _cache["mesh"] = mesh
    return _cache["fn"], _cache["mesh"]


def _stage(inputs):
    fn, mesh = _compiled()
    src_p, dstl_p, w_p = _partition_edges(np.asarray(inputs["edge_index"]))
    rep = jax.NamedSharding(mesh, P())
    shd = jax.NamedSharding(mesh, P("core"))
    xnp = np.asarray(inputs["x"], np.float32)
    args = [
        jax.device_put(xnp, rep),
        jax.device_put(xnp.reshape(CORES, NSH, D), shd),
        jax.device_put(src_p, shd),
        jax.device_put(dstl_p, shd),
        jax.device_put(w_p, shd),
    ]
    for k in ("W1l", "b1l", "W1r", "W2l", "b2l", "W2r", "Wc", "bc"):
        args.append(jax.device_put(np.asarray(inputs[k], np.float32), rep))
    return fn, args


def _kernel_host(inputs) -> np.ndarray:
    """Correctness fallback (vectorized numpy, dst-sorted reduceat)."""
    x = np.asarray(inputs["x"], np.float32)
    ei = np.asarray(inputs["edge_index"])
    src = ei[0].astype(np.int64)
    dst = ei[1].astype(np.int64)
    order = np.argsort(dst, kind="stable")
    src_s, dst_s = src[order], dst[order]
    cnt = np.bincount(dst_s, minlength=N).astype(np.float32)
    starts = np.zeros(N, np.int64)
    starts[1:] = np.cumsum(cnt.astype(np.int64))[:-1]
    nz = cnt > 0
    inv = 1.0 / np.maximum(cnt, 1.0)

    def sage(feat, Wl, bl, Wr):
        msg = feat[src_s]
        red = np.add.reduceat(msg, starts[nz], axis=0)
        # reduceat with repeated/trailing indices needs the nz mask
        agg = np.zeros_like(feat)
        agg[nz] = red[: nz.sum()] if False else red
        agg *= inv[:, None]
        return agg @ np.asarray(Wl, np.float32).T + np.asarray(bl, np.float32) \
            + feat @ np.asarray(Wr, np.float32).T

    h = np.maximum(sage(x, inputs["W1l"], inputs["b1l"], inputs["W1r"]), 0.0)
    z = sage(h, inputs["W2l"], inputs["b2l"], inputs["W2r"])
    return (z @ np.asarray(inputs["Wc"], np.float32).T
            + np.asarray(inputs["bc"], np.float32)).astype(np.float32)


def kernel(**inputs) -> np.ndarray:
    try:
        fn, args = _stage(inputs)
        out = fn(*args)
        return np.asarray(jax.block_until_ready(out))
    except Exception as e:  # device path unavailable -> host fallback
        import sys
        print(f"kernel: device path failed ({type(e).__name__}), "
              f"using host fallback", file=sys.stderr)
        return _kernel_host(inputs)


def timed_kernel_ns(inputs, n1=6, n2=12):
    """Device execution time via pipelined-dispatch slope (launch overhead
    cancels): T = (total(n2) - total(n1)) / (n2 - n1)."""
    import time
    fn, args = _stage(inputs)
    o = fn(*args); jax.block_until_ready(o)

    def total(n):
        t0 = time.perf_counter()
        for _ in range(n):
            o = fn(*args)
        jax.block_until_ready(o)
        return time.perf_counter() - t0

    t1 = min(total(n1) for _ in range(3))
    t2 = min(total(n2) for _ in range(3))
    return max((t2 - t1) / (n2 - n1), 0.0) * 1e9
